# revision 26
# baseline (speedup 1.0000x reference)
"""Trainium2 Bass kernel for nn_DampedMultidimEMACumsum.

Strategy
--------
Data-parallel over B: core b processes x[b] ([N=4096, C=512]) end to end.

The reference computes, per (c, h):
    z = A*x*exp(-E), S = cumsum(z), hid = S*exp(E), E[t] = clip(t*log r, -60, 60)
which is algebraically identical (verified to ~2e-5 rel) to the recurrence
    hid[t] = rho[t]*hid[t-1] + A*x[t],   rho[t] = exp(E[t] - E[t-1])
so on device we run `tensor_tensor_scan` (fp32 state) along the free/time axis
with channels-on-partitions, with the host-precomputed rho table streamed in.
A is folded into the output weights (etaA = eta*A), so the scan consumes x
directly.  y = sum_h etaA*hid, then out = y_f @ wf.T + y_b @ wb.T + bias via PE
matmuls (bias added with a K=1 ones-row matmul into PSUM).

Pass 1 scans forward time, keeping y_f resident in SBUF; pass 2 scans reversed
time (negative-stride APs) and fuses the projection + output DMA per 128-row
time block.
"""

import os
import sys
import numpy as np

for _p in ("/opt/trn_rl_repo", "/root/.axon_site/_ro/trn_rl_repo"):
    if os.path.isdir(_p) and _p not in sys.path:
        sys.path.append(_p)

import concourse.bass as bass
import concourse.tile as tile
from concourse import mybir
from concourse.bass_utils import run_bass_kernel_spmd

F32 = mybir.dt.float32


def _install_wait_split():
    """Split multi-wait instructions in serialized BIR before walrus.

    The walrus build here rejects instructions whose ISA encoding would carry
    more than one sync-wait ("Too many sync wait commands"), while Tile
    freely assigns several.  Move all but one wait of each instruction onto
    standalone single-wait EventSemaphore instructions inserted just before
    it on the same engine — semantically identical (the sequencer blocks on
    them in order).
    """
    import orjson
    import concourse.bass_utils as bu
    import concourse.bass2jax as b2j
    if getattr(bu, "_wait_split_installed", False):
        return

    ctr = [0]

    def split_multi_waits(bir_bytes, max_waits=1):
        d = orjson.loads(bir_bytes)
        changed = False
        for fn in d.get("functions", []):
            for bb in fn.get("blocks", []):
                new_insts = []
                for ins in bb.get("instructions", []):
                    si = ins.get("sync_info") or {}
                    waits = si.get("on_wait") or []
                    if len(waits) > max_waits:
                        for w in waits[:-max_waits]:
                            ctr[0] += 1
                            new_insts.append({
                                "debug": ins.get("debug", 0),
                                "engine": ins["engine"],
                                "ins": [], "outs": [],
                                "name": f"I-wsplit{ctr[0]}",
                                "opcode": "EventSemaphore",
                                "sync_info": {"on_update": [],
                                              "on_wait": [w]},
                            })
                        si["on_wait"] = waits[-max_waits:]
                        ins["sync_info"] = si
                        changed = True
                    new_insts.append(ins)
                bb["instructions"] = new_insts
        return orjson.dumps(d) if changed else bir_bytes

    orig = bu.compile_bir_kernel

    def patched(bir_json, tmpdir, neff_name="file.neff"):
        return orig(split_multi_waits(bir_json), tmpdir, neff_name=neff_name)

    bu.compile_bir_kernel = patched
    b2j.compile_bir_kernel = patched
    bu._wait_split_installed = True


_install_wait_split()

B, N, C, H = 8, 4096, 512, 4
TCH = 512                       # time chunk for scan/etared tiles


F32R = mybir.dt.float32r
_PROJ_F32R = os.environ.get("PROJ_F32R", "1") == "1"


def _build_nc(N=N, C=C, H=H, TCH=TCH):
    """Single pass: per chunk i, forward scans of chunk i and backward scans
    of chunk NTC-1-i share one rho chunk DMA.  Projection (fp32r matmuls)
    fires as soon as both directions of a time chunk are complete."""
    NCB = C // 128
    NTC = N // TCH
    TB = TCH // 128

    nc = bass.Bass()
    x_d = nc.declare_dram_parameter("x", [N, C], F32, isOutput=False)
    # rho layout [C, H, N]: per (cblk) one DMA of [128, H, TCH], contiguous rows
    rho_d = nc.declare_dram_parameter("rho", [C, H, N], F32, isOutput=False)
    etaA_d = nc.declare_dram_parameter("etaA", [C, H], F32, isOutput=False)
    wfT_d = nc.declare_dram_parameter("wfT", [C, C], F32, isOutput=False)
    wbT_d = nc.declare_dram_parameter("wbT", [C, C], F32, isOutput=False)
    wkf_d = nc.declare_dram_parameter("wkf", [C, C], F32, isOutput=False)
    wkb_d = nc.declare_dram_parameter("wkb", [C, C], F32, isOutput=False)
    brow_d = nc.declare_dram_parameter("brow", [1, C], F32, isOutput=False)
    ones_d = nc.declare_dram_parameter("ones", [1, 128], F32, isOutput=False)
    ident_d = nc.declare_dram_parameter("ident", [128, 128], F32, isOutput=False)
    out_d = nc.declare_dram_parameter("out", [N, C], F32, isOutput=True)

    from contextlib import ExitStack
    with tile.TileContext(nc) as tc, ExitStack() as ctx:
        consts = ctx.enter_context(tc.tile_pool(name="consts", bufs=1))
        yfp = ctx.enter_context(tc.tile_pool(name="yf", bufs=1))
        ybp = ctx.enter_context(tc.tile_pool(name="yb", bufs=1))
        xtp = ctx.enter_context(tc.tile_pool(name="xt", bufs=2))
        xinp = ctx.enter_context(tc.tile_pool(name="xin", bufs=4))
        gp = ctx.enter_context(tc.tile_pool(name="g", bufs=1))
        rhop = ctx.enter_context(tc.tile_pool(name="rho", bufs=2))
        outp = ctx.enter_context(tc.tile_pool(name="out", bufs=3))
        carp = ctx.enter_context(tc.tile_pool(name="carry", bufs=1))
        psp = ctx.enter_context(tc.tile_pool(name="psum", bufs=3, space="PSUM"))
        pst = ctx.enter_context(tc.tile_pool(name="psumt", bufs=2, space="PSUM"))
        psvp = ctx.enter_context(tc.tile_pool(name="psumv", bufs=1, space="PSUM"))

        etaA = consts.tile([128, NCB * H], F32, name="etaA")
        for cb in range(NCB):
            nc.sync.dma_start(etaA[:, cb * H:(cb + 1) * H],
                              etaA_d[cb * 128:(cb + 1) * 128, :])
        wdt = F32R if _PROJ_F32R else F32
        wf = consts.tile([128, NCB * C], wdt, name="wf")
        wb = consts.tile([128, NCB * C], wdt, name="wb")
        for cb in range(NCB):
            for (wt, wsrc) in ((wf, wfT_d), (wb, wbT_d)):
                wstg = xinp.tile([128, C], F32, name="wstg", tag="xin")
                nc.sync.dma_start(wstg[:], wsrc[cb * 128:(cb + 1) * 128, :])
                nc.vector.tensor_copy(wt[:, cb * C:(cb + 1) * C], wstg[:])
        brow = consts.tile([1, C], F32, name="brow")
        nc.sync.dma_start(brow[:], brow_d[:])
        ones = consts.tile([1, 128], F32, name="ones")
        nc.sync.dma_start(ones[:], ones_d[:])
        ident = consts.tile([128, 128], F32, name="ident")
        nc.sync.dma_start(ident[:], ident_d[:])

        # one-time bias broadcast: b_bc[p, j] = brow[j] via ones.T @ brow
        b_bc = consts.tile([128, C], F32, name="b_bc")
        ps_b = psp.tile([128, C], F32, name="ps_b", tag="ps")
        nc.tensor.matmul(ps_b[:], ones[:], brow[:], start=True, stop=True)
        nc.scalar.activation(b_bc[:], ps_b[:],
                             mybir.ActivationFunctionType.Copy)

        # full-length per-direction results, resident (4 x 16KB/partition each)
        ydt = F32R if _PROJ_F32R else F32
        yf = [yfp.tile([128, N], ydt, name=f"yf{cb}") for cb in range(NCB)]
        yb = [ybp.tile([128, N], ydt, name=f"yb{cb}") for cb in range(NCB)]
        carF, carB = {}, {}
        for cb in range(NCB):
            for h in range(H):
                carF[(cb, h)] = carp.tile([128, 1], F32, name=f"cf{cb}_{h}")
                carB[(cb, h)] = carp.tile([128, 1], F32, name=f"cb{cb}_{h}")

        def load_xt(tc0, cb_needed):
            """DMA x chunk tc0 ([TCH,C]) and transpose -> per-cb [128, TCH]."""
            xts = [xtp.tile([128, TCH], F32, name=f"xt{cb}", tag=f"xt{cb}")
                   for cb in range(NCB)]
            for tb in range(TB):
                t0 = tc0 * TCH + tb * 128
                xin = xinp.tile([128, C], F32, name="xin", tag="xin")
                nc.sync.dma_start(xin[:], x_d[t0:t0 + 128, :])
                for cb in range(NCB):
                    pt = pst.tile([128, 128], F32, name="pt", tag="pt")
                    nc.tensor.transpose(pt[:], xin[:, cb * 128:(cb + 1) * 128],
                                        ident[:])
                    nc.scalar.activation(xts[cb][:, tb * 128:(tb + 1) * 128],
                                         pt[:],
                                         mybir.ActivationFunctionType.Copy)
            return xts

        def scans_and_reduce(cb, rho_ap, xt_ap, ydst_ap, car, first):
            ecol = lambda h: etaA[:, cb * H + h:cb * H + h + 1]
            for h in range(H):
                g = gp.tile([128, TCH], F32, name="g", tag="g")
                init = 0.0 if first else car[(cb, h)][:]
                nc.vector.tensor_tensor_scan(g[:], rho_ap[:, h, :], xt_ap, init,
                                             mybir.AluOpType.mult,
                                             mybir.AluOpType.add)
                nc.vector.tensor_copy(car[(cb, h)][:], g[:, TCH - 1:TCH])
                if h == 0:
                    nc.vector.tensor_scalar(ydst_ap, g[:], ecol(0), None,
                                            mybir.AluOpType.mult)
                else:
                    nc.vector.scalar_tensor_tensor(ydst_ap, g[:], ecol(h),
                                                   ydst_ap,
                                                   mybir.AluOpType.mult,
                                                   mybir.AluOpType.add)

        def project_chunk(j):
            """Projection + bias + store for time chunk j (both dirs ready)."""
            for tb in range(TB):
                t0 = j * TCH + tb * 128
                ps = psp.tile([128, C], F32, name="ps", tag="ps")
                for cb in range(NCB):
                    nc.tensor.matmul(ps[:], yf[cb][:, t0:t0 + 128],
                                     wf[:, cb * C:(cb + 1) * C],
                                     start=(cb == 0), stop=False)
                    nc.tensor.matmul(ps[:], yb[cb][:, t0:t0 + 128],
                                     wb[:, cb * C:(cb + 1) * C],
                                     start=False, stop=(cb == NCB - 1))
                osb = outp.tile([128, C], F32, name="osb", tag="osb")
                nc.vector.tensor_tensor(osb[:], ps[:], b_bc[:],
                                        mybir.AluOpType.add)
                nc.sync.dma_start(out_d[t0:t0 + 128, :], osb[:])

        for i in range(NTC):
            jb = NTC - 1 - i          # backward real-time chunk
            xts_f = load_xt(i, None)
            xts_b = xts_f if jb == i else load_xt(jb, None)
            for cb in range(NCB):
                rho_t = rhop.tile([128, H, TCH], F32, name="rhoc",
                                  tag="rhoc")
                nc.sync.dma_start(
                    rho_t[:],
                    rho_d[cb * 128:(cb + 1) * 128, :, i * TCH:(i + 1) * TCH])
                scans_and_reduce(cb, rho_t, xts_f[cb][:],
                                 yf[cb][:, i * TCH:(i + 1) * TCH],
                                 carF, first=(i == 0))
                scans_and_reduce(cb, rho_t, xts_b[cb][:, ::-1],
                                 yb[cb][:, jb * TCH:(jb + 1) * TCH][:, ::-1],
                                 carB, first=(i == 0))
            # chunks complete after this step: j <= i (fwd) and j >= jb (bwd)
            lo, hi = max(jb, 0), i
            prev_lo, prev_hi = NTC - i, i - 1   # completed before this step
            for j in range(lo, hi + 1):
                if prev_lo <= j <= prev_hi:
                    continue
                project_chunk(j)
    return nc


def build_v3(N, C, H, TCH, Ks):
    NCB = C // 128
    NTC = N // TCH
    TB = TCH // 128
    Ks = list(Ks)
    assert len(Ks) == NCB and all(1 <= k <= NTC for k in Ks)
    xtiles = [j for j in range(NCB) if Ks[j] < NTC]   # tiles needing X

    nc = bass.Bass()
    x_d = nc.declare_dram_parameter("x", [N, C], F32, isOutput=False)
    rho_d = nc.declare_dram_parameter("rho", [C, H, N], F32, isOutput=False)
    etaA_d = nc.declare_dram_parameter("etaA", [C, H], F32, isOutput=False)
    kap_d = nc.declare_dram_parameter("kap", [C, 1], F32, isOutput=False)
    wfT_d = nc.declare_dram_parameter("wfT", [C, C], F32, isOutput=False)
    wbT_d = nc.declare_dram_parameter("wbT", [C, C], F32, isOutput=False)
    wkf_d = nc.declare_dram_parameter("wkf", [C, C], F32, isOutput=False)
    wkb_d = nc.declare_dram_parameter("wkb", [C, C], F32, isOutput=False)
    brow_d = nc.declare_dram_parameter("brow", [1, C], F32, isOutput=False)
    ones_d = nc.declare_dram_parameter("ones", [1, 128], F32, isOutput=False)
    ident_d = nc.declare_dram_parameter("ident", [128, 128], F32, isOutput=False)
    out_d = nc.declare_dram_parameter("out", [N, C], F32, isOutput=True)

    from contextlib import ExitStack
    with tile.TileContext(nc) as tc, ExitStack() as ctx:
        consts = ctx.enter_context(tc.tile_pool(name="consts", bufs=1))
        xbp = ctx.enter_context(tc.tile_pool(name="xb", bufs=1))
        ytp = ctx.enter_context(tc.tile_pool(name="yt", bufs=1))
        xtp = ctx.enter_context(tc.tile_pool(name="xt", bufs=2))
        xinp = ctx.enter_context(tc.tile_pool(name="xin", bufs=4))
        gp = ctx.enter_context(tc.tile_pool(name="g", bufs=2))
        rhop = ctx.enter_context(tc.tile_pool(name="rho", bufs=2))
        outp = ctx.enter_context(tc.tile_pool(name="out", bufs=3))
        carp = ctx.enter_context(tc.tile_pool(name="carry", bufs=1))
        psp = ctx.enter_context(tc.tile_pool(name="psum", bufs=3, space="PSUM"))
        pst = ctx.enter_context(tc.tile_pool(name="psumt", bufs=2, space="PSUM"))
        psvp = ctx.enter_context(tc.tile_pool(name="psumv", bufs=1, space="PSUM"))

        # ---------------- constants ----------------
        etaA = consts.tile([128, NCB * H], F32, name="etaA")
        kap = consts.tile([128, NCB], F32, name="kap")
        nkap = consts.tile([128, NCB], F32, name="nkap")
        for j in range(NCB):
            nc.sync.dma_start(etaA[:, j * H:(j + 1) * H],
                              etaA_d[j * 128:(j + 1) * 128, :])
            nc.sync.dma_start(kap[:, j:j + 1], kap_d[j * 128:(j + 1) * 128, :])
        nc.vector.tensor_scalar(nkap[:], kap[:], -1.0, None, mybir.AluOpType.mult)
        wf = consts.tile([128, NCB * C], F32R, name="wf")
        wb = consts.tile([128, NCB * C], F32R, name="wb")
        NXT = max(len(xtiles), 1)
        xpos = {j: i for i, j in enumerate(xtiles)}
        wkf = consts.tile([128, NXT * C], F32R, name="wkf")
        wkb = consts.tile([128, NXT * C], F32R, name="wkb")
        for j in range(NCB):
            pairs = [(wf, wfT_d, j), (wb, wbT_d, j)]
            if j in xpos:
                pairs += [(wkf, wkf_d, xpos[j]), (wkb, wkb_d, xpos[j])]
            for (wt, wsrc, col) in pairs:
                wstg = xinp.tile([128, C], F32, name="wstg", tag="xin")
                nc.sync.dma_start(wstg[:], wsrc[j * 128:(j + 1) * 128, :])
                nc.vector.tensor_copy(wt[:, col * C:(col + 1) * C], wstg[:])
        brow_f = consts.tile([1, C], F32, name="brow_f")
        nc.sync.dma_start(brow_f[:], brow_d[:])
        ones_f = consts.tile([1, 128], F32, name="ones_f")
        nc.sync.dma_start(ones_f[:], ones_d[:])
        brow = consts.tile([1, C], F32R, name="brow")
        ones = consts.tile([1, 128], F32R, name="ones")
        nc.vector.tensor_copy(brow[:], brow_f[:])
        nc.vector.tensor_copy(ones[:], ones_f[:])
        ident = consts.tile([128, 128], F32, name="ident")
        nc.sync.dma_start(ident[:], ident_d[:])
        ones_t = consts.tile([128, TCH], F32, name="ones_t")
        nc.gpsimd.memset(ones_t[:], 1.0)

        # ---------------- state ----------------
        # X buffers: [128, N+1]; col0 = 0, chunk c occupies cols [1+c*T, 1+(c+1)*T)
        Xb = {j: xbp.tile([128, N + 1], F32R, name=f"Xb{j}") for j in xtiles}
        for j in xtiles:
            nc.gpsimd.memset(Xb[j][:, 0:1].bitcast(F32), 0.0)
        # transient y storage
        yf_tr = {(j, c): ytp.tile([128, TCH], F32R, name=f"yftr{j}_{c}")
                 for j in range(NCB) for c in range(Ks[j])}
        yb_tr = {(j, p): ytp.tile([128, TCH], F32R, name=f"ybtr{j}_{p}")
                 for j in range(NCB) for p in range(Ks[j])}
        offF = {j: carp.tile([128, 1], F32, name=f"offF{j}") for j in xtiles}
        offB = {j: carp.tile([128, 1], F32, name=f"offB{j}") for j in xtiles}
        vF = {j: consts.tile([1, C], F32, name=f"vF{j}") for j in xtiles}
        vB = {j: consts.tile([1, C], F32, name=f"vB{j}") for j in xtiles}
        Rrow = consts.tile([1, C], F32R, name="Rrow")

        def off_to_v(offcol, wtile, j, vdst):
            """vdst[1,C] = offcol.T @ W_tile (K=128 matmul, M=1)."""
            offr = carp.tile([128, 1], F32R, name=f"offr{j}_{vdst.name}")
            nc.vector.tensor_copy(offr[:], offcol)
            psv = psvp.tile([1, C], F32, name="psv", tag="psv")
            nc.tensor.matmul(psv[:], offr[:], wtile[:, j * C:(j + 1) * C],
                             start=True, stop=True)
            nc.scalar.activation(vdst[:], psv[:],
                                 mybir.ActivationFunctionType.Copy)
        carF, carB = {}, {}
        for j in range(NCB):
            for h in range(H):
                carF[(j, h)] = carp.tile([128, 1], F32, name=f"cf{j}_{h}")
                carB[(j, h)] = carp.tile([128, 1], F32, name=f"cb{j}_{h}")

        def load_xt(c, need=None):
            need = list(range(NCB)) if need is None else need
            xts = {}
            for j in need:
                xts[j] = xtp.tile([128, TCH], F32, name=f"xt{j}", tag=f"xt{j}")
            xins = []
            cw = [min(need), max(need)]
            c0, c1 = cw[0] * 128, (cw[1] + 1) * 128
            for tb in range(TB):
                t0 = c * TCH + tb * 128
                xin = xinp.tile([128, c1 - c0], F32, name="xin", tag="xin")
                nc.sync.dma_start(xin[:], x_d[t0:t0 + 128, c0:c1])
                xins.append(xin)
            for j in need:
                pt = pst.tile([128, TCH], F32, name="pt", tag="pt")
                for tb in range(TB):
                    nc.tensor.transpose(
                        pt[:, tb * 128:(tb + 1) * 128],
                        xins[tb][:, j * 128 - c0:(j + 1) * 128 - c0],
                        ident[:])
                nc.scalar.activation(xts[j][:], pt[:],
                                     mybir.ActivationFunctionType.Copy)
            return xts

        def transient(j, c, xt_ap, ydst_ap, car, first):
            """4 EMA scans + eta-reduce for tile j over rho chunk c."""
            rho_t = rhop.tile([128, H, TCH], F32, name="rhoc", tag="rhoc")
            nc.sync.dma_start(
                rho_t[:], rho_d[j * 128:(j + 1) * 128, :, c * TCH:(c + 1) * TCH])
            ecol = lambda h: etaA[:, j * H + h:j * H + h + 1]
            for h in range(H):
                g = gp.tile([128, TCH], F32, name="g", tag="g")
                init = 0.0 if first else car[(j, h)][:]
                nc.vector.tensor_tensor_scan(g[:], rho_t[:, h, :], xt_ap, init,
                                             mybir.AluOpType.mult,
                                             mybir.AluOpType.add)
                nc.scalar.activation(car[(j, h)][:], g[:, TCH - 1:TCH],
                                     mybir.ActivationFunctionType.Copy)
                if h == 0:
                    nc.vector.tensor_scalar(ydst_ap, g[:], ecol(0), None,
                                            mybir.AluOpType.mult)
                else:
                    nc.vector.scalar_tensor_tensor(ydst_ap, g[:], ecol(h),
                                                   ydst_ap,
                                                   mybir.AluOpType.mult,
                                                   mybir.AluOpType.add)

        # ================= phase F: ascending =================
        for c in range(NTC):
            xts = load_xt(c)
            for j in range(NCB):
                if j in Xb:
                    # X chunk scan (plain cumsum): state = 1*state + x
                    nc.vector.tensor_tensor_scan(
                        Xb[j][:, 1 + c * TCH:1 + (c + 1) * TCH],
                        ones_t[:], xts[j][:],
                        Xb[j][:, c * TCH:c * TCH + 1],
                        mybir.AluOpType.mult, mybir.AluOpType.add)
                if c < Ks[j]:
                    transient(j, c, xts[j][:], yf_tr[(j, c)][:], carF,
                              first=(c == 0))
                if j in Xb and c == Ks[j] - 1:
                    # off_f = y_f(S-1) - kappa*X(S-1); X(S-1) at col Ks*TCH
                    S = Ks[j] * TCH
                    nc.vector.scalar_tensor_tensor(
                        offF[j][:], Xb[j][:, S:S + 1], nkap[:, j:j + 1],
                        yf_tr[(j, c)][:, TCH - 1:TCH],
                        mybir.AluOpType.mult, mybir.AluOpType.add)
                    off_to_v(offF[j][:], wf, j, vF[j])

        # ================= phase B: descending =================
        # R row: bias + sum of active steady offset vectors
        nc.vector.tensor_copy(Rrow[:], brow_f[:])
        for j in xtiles:
            nc.vector.tensor_tensor(Rrow[:], Rrow[:], vF[j][:],
                                    mybir.AluOpType.add)
        for c in range(NTC - 1, -1, -1):
            p = NTC - 1 - c          # reversed-position chunk index
            needB = [j for j in range(NCB) if p < Ks[j]]
            if needB:
                xts = load_xt(c, needB)
            for j in range(NCB):
                if p < Ks[j]:
                    transient(j, p, xts[j][:, ::-1],
                              yb_tr[(j, p)][:, ::-1], carB, first=(p == 0))
                if j in Xb and p == Ks[j] - 1:
                    # off_b = ybar(S-1) + kappa*Xe(N-S);  Xe(N-S) at col N-S
                    S = Ks[j] * TCH
                    nc.vector.scalar_tensor_tensor(
                        offB[j][:], Xb[j][:, N - S:N - S + 1], kap[:, j:j + 1],
                        yb_tr[(j, p)][:, 0:1],
                        mybir.AluOpType.mult, mybir.AluOpType.add)
                    off_to_v(offB[j][:], wb, j, vB[j])
                if j in Xb and p == Ks[j]:
                    # tile j enters bwd-steady at this chunk
                    nc.vector.tensor_tensor(Rrow[:], Rrow[:], vB[j][:],
                                            mybir.AluOpType.add)
                if j in Xb and c == Ks[j] - 1:
                    # tile j leaves fwd-steady below this chunk
                    nc.vector.tensor_tensor(Rrow[:], Rrow[:], vF[j][:],
                                            mybir.AluOpType.subtract)
            # ---- projection for chunk c ----
            for tb in range(TB):
                t0 = c * TCH + tb * 128
                sl = slice(tb * 128, (tb + 1) * 128)
                ps = psp.tile([128, C], F32, name="ps", tag="ps")
                nc.tensor.matmul(ps[:], ones[:], Rrow[:], start=True, stop=False)
                for j in range(NCB):
                    if c < Ks[j]:
                        nc.tensor.matmul(ps[:], yf_tr[(j, c)][:, sl],
                                         wf[:, j * C:(j + 1) * C],
                                         start=False, stop=False)
                    else:
                        nc.tensor.matmul(
                            ps[:], Xb[j][:, 1 + t0:1 + t0 + 128],
                            wkf[:, xpos[j] * C:(xpos[j] + 1) * C],
                            start=False, stop=False)
                    if p < Ks[j]:
                        nc.tensor.matmul(ps[:], yb_tr[(j, p)][:, sl],
                                         wb[:, j * C:(j + 1) * C],
                                         start=False, stop=(j == NCB - 1))
                    else:
                        nc.tensor.matmul(
                            ps[:], Xb[j][:, t0:t0 + 128],
                            wkb[:, xpos[j] * C:(xpos[j] + 1) * C],
                            start=False, stop=(j == NCB - 1))
                osb = outp.tile([128, C], F32, name="osb", tag="osb")
                nc.scalar.activation(osb[:], ps[:],
                                     mybir.ActivationFunctionType.Copy)
                nc.sync.dma_start(out_d[t0:t0 + 128, :], osb[:])
    return nc


def analyze(alpha_logits, delta_logits, beta_logits, eta, proj_w, proj_b,
            N, TCH):
    f32 = np.float32
    sig = lambda v: (1.0 / (1.0 + np.exp(-np.asarray(v, f32), dtype=f32))).astype(f32)
    alpha, delta, beta = sig(alpha_logits), sig(delta_logits), sig(beta_logits)
    A = (alpha * beta).astype(f32)
    r = np.clip((1.0 - alpha * delta).astype(f32), f32(1e-4), f32(1.0 - 1e-4))
    lam = np.log(r, dtype=f32)                       # [C,H] < 0
    C, H = lam.shape
    NTC = N // TCH

    # first t with t*lam <= -60 (exact in fp32, same arithmetic as the table)
    t = np.arange(N, dtype=f32)[:, None, None]
    E = np.maximum(t * lam[None], f32(-60.0)).astype(f32)     # [N,C,H]
    clamped = (t * lam[None]) <= f32(-60.0)
    # t_dagger = first clamped index (N if never)
    tdag = np.where(clamped.any(0), clamped.argmax(0), N)     # [C,H]
    Tstar = tdag.max(1)                                       # [C]

    perm = np.argsort(Tstar, kind="stable")
    Ks = []
    for j in range(C // 128):
        mx = Tstar[perm[j * 128:(j + 1) * 128]].max()
        Ks.append(int(min(np.ceil((mx + 1) / TCH), NTC)))

    Eprev = np.concatenate([np.zeros((1, C, H), f32), E[:-1]], 0)
    rho = np.exp((E - Eprev).astype(f32), dtype=f32)          # [N,C,H]
    rho_chn = np.ascontiguousarray(rho.transpose(1, 2, 0)[perm])  # [C,H,N] perm

    etaA = (np.asarray(eta, f32) * A).astype(f32)[perm]       # [C,H] perm
    kap = etaA.sum(1, dtype=f32).astype(f32).reshape(C, 1)    # [C,1]
    pw = np.asarray(proj_w, f32)
    wfT = np.ascontiguousarray(pw[:, :C].T[perm])
    wbT = np.ascontiguousarray(pw[:, C:].T[perm])
    tables = {
        "rho": rho_chn,
        "etaA": np.ascontiguousarray(etaA),
        "kap": np.ascontiguousarray(kap),
        "wfT": wfT,
        "wbT": wbT,
        "wkf": np.ascontiguousarray((wfT * kap).astype(np.float32)),
        "wkb": np.ascontiguousarray((wbT * (-kap)).astype(np.float32)),
        "brow": np.asarray(proj_b, f32).reshape(1, C).copy(),
        "ones": np.ones((1, 128), f32),
        "ident": np.eye(128, dtype=f32),
    }
    return perm, Ks, tables


def _host_tables(alpha_logits, delta_logits, beta_logits, eta, proj_w, proj_b,
                 N=N):
    f32 = np.float32
    sig = lambda v: (1.0 / (1.0 + np.exp(-np.asarray(v, f32), dtype=f32))).astype(f32)
    alpha, delta, beta = sig(alpha_logits), sig(delta_logits), sig(beta_logits)
    A = (alpha * beta).astype(f32)
    r = np.clip((1.0 - alpha * delta).astype(f32), f32(1e-4), f32(1.0 - 1e-4))
    lam = np.log(r, dtype=f32)
    C_, H_ = lam.shape
    t = np.arange(N, dtype=f32)[:, None, None]
    E = np.maximum(t * lam[None], f32(-60.0)).astype(f32)
    Eprev = np.concatenate([np.zeros((1, C_, H_), f32), E[:-1]], 0)
    rho_chn = np.ascontiguousarray(
        np.exp((E - Eprev).astype(f32), dtype=f32).transpose(1, 2, 0))
    pw = np.asarray(proj_w, f32)
    C = C_
    tables = {
        "rho": rho_chn,
        "etaA": (np.asarray(eta, f32) * A).astype(f32),
        "wfT": np.ascontiguousarray(pw[:, :C].T),
        "wbT": np.ascontiguousarray(pw[:, C:].T),
        "brow": np.asarray(proj_b, f32).reshape(1, C).copy(),
        "ones": np.ones((1, 128), f32),
        "ident": np.eye(128, dtype=f32),
    }
    return tables


_NC_CACHE = {}


def _get_nc():
    if "nc" not in _NC_CACHE:
        _NC_CACHE["nc"] = _build_nc()
    return _NC_CACHE["nc"]


def run(inputs, trace=False, trace_kwargs=None):
    """Run on hardware, returning (out [B,N,C], BassKernelResults)."""
    x = np.asarray(inputs["x"], np.float32)
    assert x.shape == (B, N, C)
    perm, Ks, tables = analyze(inputs["alpha_logits"], inputs["delta_logits"],
                               inputs["beta_logits"], inputs["eta"],
                               inputs["proj_w"], inputs["proj_b"], N, TCH)
    key = ("v3", tuple(Ks))
    if _NC_CACHE.get("key") != key:
        _NC_CACHE["nc"] = build_v3(N, C, H, TCH, Ks)
        _NC_CACHE["key"] = key
    nc = _NC_CACHE["nc"]
    in_maps = [dict(tables, x=np.ascontiguousarray(x[b][:, perm]))
               for b in range(B)]
    res = run_bass_kernel_spmd(nc, in_maps, list(range(B)), trace=trace,
                               **(dict(trace_kwargs=trace_kwargs) if trace_kwargs else {}))
    out = np.stack([res.results[b]["out"] for b in range(B)], axis=0)
    return out.astype(inputs["x"].dtype, copy=False), res


def kernel(**inputs):
    out, _ = run(inputs)
    return out
